# revision 67
# baseline (speedup 1.0000x reference)
"""Domain-specific BatchNorm (nn_DSBatchNorm) Trainium2 Bass kernel.

Data-parallel over rows across 8 NeuronCores. Per core:
  pass A: segmented per-domain sums/sumsq via bf16 one-hot matmuls into
          PSUM (counts are accumulated from the one-hots with one tiny DVE
          add per chunk and folded at the end). The bf16 casts of the last
          NCACHE chunks stay in SBUF so pass B can normalize them without
          re-reading x from HBM. Casts/squares are spread over ACT/DVE/
          gpsimd so no single engine exceeds the per-chunk DMA budget.
  tiny AllReduce of the [8, 2F+1] packed stats (bf16 payload: local sums
          are fp32-exact; bf16 rounding of the partials is far inside
          tolerance and halves the collective's software processing time.
          AllGather + local reduce was tried and is slower: CC cost
          scales with moved bytes).
  table math: A = gamma*inv*nz, B = beta*nz - A*mean_e  (per-domain [8,F]),
          hi/lo bf16 split of [A|B] replicated to the four PE quadrant
          bases with two accumulating REP matmuls.
  pass B: per row-tile, two quadrant matmuls gather per-row A_rows/B_rows
          into PSUM (hi/lo bf16 split stacked along K reconstructs f32
          exactly in the accumulator); DVE computes tmp = x*A (bf16);
          out = tmp + B_rows runs never-in-place (in-place DVE ops
          measured ~2x slower), alternating DVE / gpsimd (gpsimd cannot
          read PSUM, so ACT stages its B_rows to SBUF bf16). The adds are
          emitted two tiles behind the gathers and the next chunk's
          one-hot compare + transposes + load are emitted mid-chunk, so
          no engine queue drains at chunk boundaries. The first four
          chunks are prepped BEFORE the collective so the otherwise-idle
          barrier window does pass-B warmup work and streams two loads.

DMA plumbing: x loads ride the sync HWDGE ring, out stores ride the
scalar ring, collective staging rides gpsimd.
"""

import sys

if "/opt/trn_rl_repo" not in sys.path:
    sys.path.insert(0, "/opt/trn_rl_repo")

import numpy as np

import concourse.bacc as bacc
import concourse.bass as bass
import concourse.tile as tile
from concourse import mybir
from concourse.bass_utils import run_bass_kernel_spmd

N_CORES = 8
N, F, D = 262144, 512, 8
NS = N // N_CORES  # rows per core
P = 128
T = NS // P  # row-tiles per core
CHUNK = 8  # row-tiles per DMA chunk (2 MB)
NCHUNKS = T // CHUNK
NCACHE = 10  # trailing chunks kept in SBUF as bf16 between passes
UNCACHED = NCHUNKS - NCACHE
EPS = 1e-5
f32 = mybir.dt.float32
bf16 = mybir.dt.bfloat16
i32 = mybir.dt.int32

_CACHE = {}

# test.py can flip this to get a traced run; grading path leaves it False
TRACE = False
LAST_RESULTS = None


def _build():
    AluOp = mybir.AluOpType
    nc = bacc.Bacc(
        "TRN2", target_bir_lowering=False, debug=False, num_devices=N_CORES
    )

    x = nc.dram_tensor("x", [NS, F], f32, kind="ExternalInput")
    yf = nc.dram_tensor("yf", [NS], f32, kind="ExternalInput")
    gamma = nc.dram_tensor("gamma", [D, F], f32, kind="ExternalInput")
    beta = nc.dram_tensor("beta", [D, F], f32, kind="ExternalInput")
    out = nc.dram_tensor("out", [NS, F], f32, kind="ExternalOutput")

    ident_c = nc.inline_tensor(np.eye(P, dtype=np.float32), name="ident_c")
    # REPh[d, m] = 1 iff m % 32 == d; REPl[d, m] = 1 iff m % 32 == d + 8:
    # two accumulating matmuls place hi rows at quadrant offsets 0..7 and
    # lo rows at 8..15 without partition-crossing engine ops.
    reph_np = np.zeros((D, P), dtype=np.float32)
    repl_np = np.zeros((D, P), dtype=np.float32)
    for m in range(P):
        if m % 32 < D:
            reph_np[m % 32, m] = 1.0
        elif m % 32 < 2 * D:
            repl_np[m % 32 - D, m] = 1.0
    reph_c = nc.inline_tensor(reph_np, name="reph_c")
    repl_c = nc.inline_tensor(repl_np, name="repl_c")

    # p-major row mapping: partition p, tile t <-> row p*T + t. Stats are
    # permutation-invariant and load/store/one-hot all use the same mapping,
    # so this is just a DMA-friendly tiling (16 KB contiguous per partition
    # per chunk).
    x_r = x[:].rearrange("(p t) f -> p t f", t=T)
    out_r = out[:].rearrange("(p t) f -> p t f", t=T)
    y_r = yf[:].rearrange("(p t) -> p t", t=T)

    W = 2 * F + 1

    with tile.TileContext(nc) as tc:
        with (
            tc.tile_pool(name="consts", bufs=1) as consts,
            tc.tile_pool(name="tables", bufs=1) as tables,
            tc.tile_pool(name="xc", bufs=3) as xcp,
            tc.tile_pool(name="xb", bufs=3) as xbp,
            tc.tile_pool(name="xsq", bufs=3) as xsqp,
            tc.tile_pool(name="oh", bufs=2) as ohp,
            tc.tile_pool(name="oc", bufs=3) as ocp,
            tc.tile_pool(name="oh2", bufs=4) as oh2p,
            tc.tile_pool(name="tmp", bufs=4) as tmpp,
            tc.tile_pool(name="bsb", bufs=3) as bsbp,
            tc.tile_pool(name="ohT", bufs=8) as ohTp,
            tc.tile_pool(name="dram", bufs=1, space="DRAM") as dram,
        ):
            # ---- constants ----
            ident = consts.tile([P, P], f32)
            nc.sync.dma_start(out=ident, in_=ident_c[:])
            ident_bf = consts.tile([P, P], bf16)
            nc.scalar.copy(ident_bf, ident)
            reph_f = consts.tile([D, P], f32)
            nc.sync.dma_start(out=reph_f, in_=reph_c[:])
            reph_bf = consts.tile([D, P], bf16)
            nc.scalar.copy(reph_bf, reph_f)
            repl_f = consts.tile([D, P], f32)
            nc.sync.dma_start(out=repl_f, in_=repl_c[:])
            repl_bf = consts.tile([D, P], bf16)
            nc.scalar.copy(repl_bf, repl_f)
            # iota_cd[p, k*D + d] = d  (pass-A batched one-hot compare)
            iota_cd_i = consts.tile([P, CHUNK * D], i32)
            nc.gpsimd.iota(
                iota_cd_i, pattern=[[0, CHUNK], [1, D]], base=0,
                channel_multiplier=0,
            )
            iota_cd = consts.tile([P, CHUNK * D], f32)
            nc.vector.tensor_copy(out=iota_cd, in_=iota_cd_i)
            # iota32[p, t*32 + ...]: values [0..7, 0..7] then [8..15, 8..15]
            # per tile: positions d and d+8 both match y=d (hi+lo gather),
            # positions 16..31 never match (pad to 32 so lhsT slices are
            # quadrant-aligned at 0/32/64/96)
            iota32_i32 = consts.tile([P, CHUNK * 4 * D], i32)
            nc.gpsimd.iota(
                iota32_i32, pattern=[[0, CHUNK], [D, 2], [0, 2], [1, D]],
                base=0, channel_multiplier=0,
            )
            iota32 = consts.tile([P, CHUNK * 4 * D], f32)
            nc.vector.tensor_copy(out=iota32, in_=iota32_i32)
            gam = consts.tile([D, F], f32)
            nc.sync.dma_start(out=gam, in_=gamma[:])
            bet = consts.tile([D, F], f32)
            nc.sync.dma_start(out=bet, in_=beta[:])
            ones_bf = consts.tile([P, 1], bf16)
            nc.vector.memset(ones_bf, 1.0)
            y_cols = consts.tile([P, T], f32)
            nc.sync.dma_start(out=y_cols, in_=y_r)

            # per-domain counts: accumulate the per-chunk one-hots with one
            # tiny [P, CHUNK, D] DVE add per chunk (replacing 256 per-tile
            # count matmuls on the PE); folded and cross-partition-summed
            # with a single N=1 matmul at stats-pack time
            cntCD = consts.tile([P, CHUNK, D], f32)
            cnt8b = consts.tile([P, D], bf16)
            nc.vector.memset(cntCD, 0.0)

            # bf16 copy of the last NCACHE chunks of x, written during pass A
            xcb = consts.tile([P, NCACHE, CHUNK, F], bf16)

            pack_bf = tables.tile([D, W], bf16)

            # ---- pass A: stats + bf16 tail cache ----
            with tc.tile_pool(name="stat_ps", bufs=1, space="PSUM") as statp:
                psum_ss = statp.tile([D, 2, F], f32)
                psum_cnt = statp.tile([D, 1], f32)

                for c in range(NCHUNKS):
                    xc = xcp.tile([P, CHUNK, F], f32)
                    nc.sync.dma_start(
                        out=xc, in_=x_r[:, c * CHUNK : (c + 1) * CHUNK, :]
                    )
                    ci = c - UNCACHED  # >= 0 for cached chunks
                    ysl = y_cols[:, c * CHUNK : (c + 1) * CHUNK]
                    # batched stats one-hot: [P, CHUNK, D]
                    ohs = ohp.tile([P, CHUNK, D], bf16)
                    ybcd = bass.AP(
                        tensor=ysl.tensor, offset=ysl.offset,
                        ap=list(ysl.ap) + [[0, D]],
                    )
                    nc.vector.tensor_tensor(
                        ohs, iota_cd.rearrange("p (k d) -> p k d", d=D), ybcd,
                        AluOp.is_equal,
                    )
                    nc.vector.tensor_tensor(cntCD, cntCD, ohs, AluOp.add)
                    for k in range(CHUNK):
                        t = c * CHUNK + k
                        if ci >= 0:
                            xb = xcb[:, ci, k, :]
                        else:
                            xb = xbp.tile([P, F], bf16)
                        # casts and squares spread over ACT/DVE/gpsimd so
                        # no engine exceeds the per-chunk DMA budget
                        if k < 5:
                            nc.scalar.copy(xb, xc[:, k, :])
                        else:
                            nc.vector.tensor_copy(out=xb, in_=xc[:, k, :])
                        xsq = xsqp.tile([P, F], bf16)
                        if k < 4:
                            nc.vector.tensor_tensor(xsq, xb, xb, AluOp.mult)
                        elif k < 6:
                            nc.scalar.square(xsq, xb)
                        else:
                            nc.gpsimd.tensor_tensor(xsq, xb, xb, AluOp.mult)
                        first = t == 0
                        last = t == T - 1
                        oh = ohs[:, k, :]
                        nc.tensor.matmul(
                            psum_ss[:, 0, :], oh, xb,
                            start=first, stop=last, skip_group_check=True,
                        )
                        nc.tensor.matmul(
                            psum_ss[:, 1, :], oh, xsq,
                            start=first, stop=last, skip_group_check=True,
                        )

                # ---- pack stats (bf16 cast fused into the PSUM copy) ----
                step = CHUNK // 2
                while step >= 1:
                    nc.vector.tensor_tensor(
                        cntCD[:, 0:step, :], cntCD[:, 0:step, :],
                        cntCD[:, step : 2 * step, :], AluOp.add,
                    )
                    step //= 2
                nc.vector.tensor_copy(out=cnt8b, in_=cntCD[:, 0, :])
                nc.tensor.matmul(
                    psum_cnt, cnt8b, ones_bf,
                    start=True, stop=True, skip_group_check=True,
                )
                nc.scalar.copy(pack_bf[:, 0:F], psum_ss[:, 0, :])
                nc.scalar.copy(pack_bf[:, F : 2 * F], psum_ss[:, 1, :])
                nc.scalar.copy(pack_bf[:, 2 * F : W], psum_cnt)

            # ---- pass-B chunk order ----
            # uncached (HBM re-read) interleaved with cached so the DMA
            # rings and engines stay jointly busy
            cached = list(range(UNCACHED, NCHUNKS))
            uncached = list(range(2, UNCACHED))
            order = list(range(min(2, UNCACHED)))
            stride = max(1, len(uncached) // max(1, len(cached)))
            while cached or uncached:
                take = uncached[:stride]
                del uncached[:stride]
                order.extend(take)
                if cached:
                    order.append(cached.pop(0))

            with (
                tc.tile_pool(name="pA", bufs=3, space="PSUM") as pAp,
                tc.tile_pool(name="pO", bufs=4, space="PSUM") as pOp,
                tc.tile_pool(name="pT", bufs=1, space="PSUM") as pTp,
            ):
                # prep(c): load (if uncached) + one-hot compare + the two
                # PE transposes. Needs no collective result, so the first
                # two chunks are prepped during the AllReduce window.
                st = {}

                def prep(c):
                    ci = c - UNCACHED
                    if ci >= 0:
                        xsrc = xcb[:, ci, :, :]
                    else:
                        xc = xcp.tile([P, CHUNK, F], f32, name="xc")
                        nc.sync.dma_start(
                            out=xc,
                            in_=x_r[:, c * CHUNK : (c + 1) * CHUNK, :],
                        )
                        xsrc = xc
                    ohs2 = oh2p.tile([P, CHUNK * 4 * D], bf16, name="ohs2")
                    ysl = y_cols[:, c * CHUNK : (c + 1) * CHUNK]
                    ybc = bass.AP(
                        tensor=ysl.tensor, offset=ysl.offset,
                        ap=list(ysl.ap) + [[0, 4 * D]],
                    )
                    nc.vector.tensor_tensor(
                        ohs2.rearrange("p (k r) -> p k r", r=4 * D),
                        iota32.rearrange("p (k r) -> p k r", r=4 * D),
                        ybc,
                        AluOp.is_equal,
                    )
                    ohTs = []
                    for h in range(CHUNK // 4):
                        psum_oT = pTp.tile([P, P], f32, name="pT")
                        nc.tensor.matmul(
                            psum_oT,
                            ohs2[:, h * P : (h + 1) * P],
                            ident_bf,
                            start=True, stop=True, skip_group_check=True,
                        )
                        ohT = ohTp.tile([P, P], bf16, name="ohT")
                        nc.scalar.copy(ohT, psum_oT)
                        ohTs.append(ohT)
                    st[c] = (xsrc, ohTs)

                for pc in order[: min(4, len(order))]:
                    prep(pc)

            # ---- stats exchange: AllReduce, bf16 payload ----
                cc_in = dram.tile([D, W], bf16)
                cc_out = dram.tile([D, W], bf16)
                nc.gpsimd.dma_start(out=cc_in, in_=pack_bf)
                nc.gpsimd.collective_compute(
                    "AllReduce",
                    AluOp.add,
                    replica_groups=[list(range(N_CORES))],
                    ins=[cc_in.opt()],
                    outs=[cc_out.opt()],
                )
                red = tables.tile([D, W], bf16)
                nc.gpsimd.dma_start(out=red, in_=cc_out)
                S = red[:, 0:F]
                Q = red[:, F : 2 * F]
                cnt = red[:, 2 * F : W]

                # ---- table math (all [8, F] / [8, 1]) ----
                cntf = tables.tile([D, 1], f32)
                nc.vector.tensor_copy(out=cntf, in_=cnt)
                safe = tables.tile([D, 1], f32)
                nc.vector.tensor_scalar(safe, cntf, 1.0, None, AluOp.max)
                rn = tables.tile([D, 1], f32)
                nc.vector.reciprocal(rn, safe)
                mb = tables.tile([D, 1], f32)
                nc.vector.tensor_scalar(mb, cntf, 1.0, None, AluOp.is_gt)
                omb = tables.tile([D, 1], f32)
                nc.vector.tensor_scalar(
                    omb, mb, -1.0, 1.0, AluOp.mult, AluOp.add
                )
                nz = tables.tile([D, 1], f32)
                nc.vector.tensor_scalar(nz, cntf, 0.0, None, AluOp.is_gt)
                eps_t = tables.tile([D, 1], f32)
                nc.vector.memset(eps_t, EPS)
                mean = tables.tile([D, F], f32)
                nc.vector.tensor_scalar(mean, S, rn, None, AluOp.mult)
                var = tables.tile([D, F], f32)
                nc.vector.tensor_scalar(var, Q, rn, None, AluOp.mult)
                m2 = tables.tile([D, F], f32)
                nc.vector.tensor_tensor(m2, mean, mean, AluOp.mult)
                nc.vector.tensor_tensor(var, var, m2, AluOp.subtract)
                # inv = 1/sqrt(var + eps), blended to 1 where count <= 1.
                # approx reciprocal: ~18-bit accurate, ~5x faster than the
                # exact DVE reciprocal on [D, F]. m2 is dead, reuse for sd.
                sd = m2
                nc.scalar.activation(
                    sd, var, mybir.ActivationFunctionType.Sqrt,
                    bias=eps_t[:, 0:1],
                )
                inv = tables.tile([D, F], f32)
                nc.vector.reciprocal_approx_fast(out=inv, in_=sd)
                nc.vector.tensor_scalar(
                    inv, inv, mb, omb, AluOp.mult, AluOp.add
                )
                # AB = [A | B]: A = gamma*inv*nz, B = beta*nz - A*mean_e
                AB = tables.tile([D, 2 * F], f32)
                A = AB[:, 0:F]
                B = AB[:, F : 2 * F]
                nc.vector.scalar_tensor_tensor(
                    A, gam, nz, inv, AluOp.mult, AluOp.mult
                )
                me = tables.tile([D, F], f32)
                nc.vector.tensor_scalar(me, mean, mb, None, AluOp.mult)
                nc.vector.tensor_tensor(me, A, me, AluOp.mult)  # A * mean_e
                nc.vector.scalar_tensor_tensor(
                    B, bet, nz, me, AluOp.mult, AluOp.subtract
                )

                # ---- hi/lo bf16 split of [A|B], quadrant-replicated ----
                hi_bf = tables.tile([D, 2 * F], bf16)
                hi32 = tables.tile([D, 2 * F], f32)
                lo_bf = tables.tile([D, 2 * F], bf16)
                nc.scalar.copy(hi_bf, AB)
                nc.scalar.copy(hi32, hi_bf)
                nc.vector.tensor_tensor(lo_bf, AB, hi32, AluOp.subtract)
                ABHL = tables.tile([P, 2 * F], bf16)
                # borrow one pA and one pO bank for the two REP halves
                # (matmul dests cannot span PSUM banks: N <= 512 each)
                pRs = [
                    pAp.tile([P, F], f32, name="pA"),
                    pOp.tile([P, F], f32, name="pO"),
                ]
                for half in range(2):
                    sl = slice(half * F, (half + 1) * F)
                    nc.tensor.matmul(
                        pRs[half], reph_bf, hi_bf[:, sl],
                        start=True, stop=False, skip_group_check=True,
                    )
                    nc.tensor.matmul(
                        pRs[half], repl_bf, lo_bf[:, sl],
                        start=False, stop=True, skip_group_check=True,
                    )
                    nc.scalar.copy(ABHL[:, sl], pRs[half])

                # ---- pass B: normalize, software-pipelined ----
                # per tile: two quadrant gathers -> DVE mult -> PE acc ->
                # ACT copy; acc/copy lag the gathers by 2 tiles; the next
                # chunk's prep is emitted mid-chunk so no engine drains at
                # chunk boundaries.
                LAG = 0
                for idx, c in enumerate(order):
                    xsrc, ohTs = st.pop(c)
                    ocs = [None, None]
                    pOs = [None] * CHUNK
                    tmps = [None] * CHUNK

                    def gather(k):
                        h, l = divmod(k, 4)
                        lhs = ohTs[h][l * 32 : (l + 1) * 32, :]
                        pA = pAp.tile([P, F], f32, name="pA")
                        nc.tensor.matmul(
                            pA, lhs, ABHL[l * 32 : (l + 1) * 32, 0:F],
                            start=True, stop=True, skip_group_check=True,
                            tile_position=(l * 32, 0),
                        )
                        pO = pOp.tile([P, F], f32, name="pO")
                        nc.tensor.matmul(
                            pO, lhs, ABHL[l * 32 : (l + 1) * 32, F : 2 * F],
                            start=True, stop=True, skip_group_check=True,
                            tile_position=(l * 32, 0),
                        )
                        tmp = tmpp.tile([P, F], bf16, name="tmp")
                        nc.vector.tensor_tensor(
                            tmp, xsrc[:, k, :], pA, AluOp.mult
                        )
                        pOs[k] = pO
                        tmps[k] = tmp

                    def finish(k):
                        # out = tmp + B_rows, never in-place (in-place DVE
                        # ops measured ~2x slower); adds alternate DVE /
                        # gpsimd (gpsimd cannot read PSUM, so ACT stages
                        # B_rows to SBUF bf16 for its tiles)
                        h2, k2 = divmod(k, CHUNK // 2)
                        if k2 == 0:
                            ocs[h2] = ocp.tile(
                                [P, CHUNK // 2, F], f32, name="oc"
                            )
                        ock = ocs[h2][:, k2, :]
                        if k % 2 == 0:
                            nc.vector.tensor_tensor(
                                ock, tmps[k], pOs[k], AluOp.add
                            )
                        else:
                            bsb = bsbp.tile([P, F], bf16, name="bsb")
                            nc.scalar.copy(bsb, pOs[k])
                            nc.gpsimd.tensor_tensor(
                                ock, tmps[k], bsb, AluOp.add
                            )
                        if k2 == CHUNK // 2 - 1:
                            base = c * CHUNK + h2 * (CHUNK // 2)
                            nc.gpsimd.dma_start(
                                out=out_r[:, base : base + CHUNK // 2, :],
                                in_=ocs[h2],
                            )

                    for k in range(CHUNK):
                        gather(k)
                        if k == 4 and idx + 4 < len(order):
                            prep(order[idx + 4])
                        if k >= LAG:
                            finish(k - LAG)
                    for k in range(CHUNK - LAG, CHUNK):
                        finish(k)

    nc.finalize()
    return nc


def _get_nc():
    if "nc" not in _CACHE:
        _CACHE["nc"] = _build()
    return _CACHE["nc"]


def kernel(x, y, gamma, beta):
    global LAST_RESULTS
    x = np.ascontiguousarray(np.asarray(x), dtype=np.float32)
    yf = np.ascontiguousarray(np.asarray(y).astype(np.float32))
    gamma = np.ascontiguousarray(np.asarray(gamma), dtype=np.float32)
    beta = np.ascontiguousarray(np.asarray(beta), dtype=np.float32)

    nc = _get_nc()
    in_maps = [
        {
            "x": x[i * NS : (i + 1) * NS],
            "yf": yf[i * NS : (i + 1) * NS],
            "gamma": gamma,
            "beta": beta,
        }
        for i in range(N_CORES)
    ]
    res = run_bass_kernel_spmd(nc, in_maps, core_ids=list(range(N_CORES)), trace=TRACE)
    LAST_RESULTS = res
    return np.concatenate([res.results[i]["out"] for i in range(N_CORES)], axis=0)
